# revision 4
# baseline (speedup 1.0000x reference)
"""NT-Xent contrastive loss (B=4096, D=256, T=0.2) on 8 Trainium2 NeuronCores.

v2: fp8 DoubleRow matmuls + pipelined preamble.

Per core (data-parallel over rows of Z = concat([z_i, z_j]); host rotates
Z so each core's 1024-row block sits at rows [0, 1024)):
  1. DMA the full [8192, 256] fp32 matrix in 8 groups of 1024 rows.
  2. Per group, on DVE only (keeps ACT free for the main loop):
     ss = row sums of squares via tensor_scalar(pow 2, accum_out);
     inv8 = 8/||e|| via pow(-0.5) (fallback: quake rsqrt + Newton);
     znat8 = fp8e4(e * inv8)  (tensor_scalar 2x_2p).
  3. Pair-transpose znat8 (viewed as uint16) into ztp[p, c, i] =
     z^T[2p+i, c] -- the DoubleRow K=256-in-one-pass layout.
  4. Main loop over 4 column-gpairs x 8 m-chunks: one [128,2048] PSUM
     tile per (gp, m) filled by 4 DoubleRow matmuls (107 ns each), then
     ACT exp(psum * 5/64) with accum_out giving row sums.
  5. denom = sum - exp(5); loss partial via ln + positive-pair dots
     (raw fp32, exact); ones-matmul partition reduce; host sums 8 scalars.
"""

import os
import sys

sys.path.insert(0, "/opt/trn_rl_repo")

import numpy as np

import concourse.bass as bass  # noqa: F401  (registers AP machinery)
import concourse.tile as tile
import concourse.mybir as mybir
from concourse import bacc, bass_utils

N_CORES = 8
B = 4096                 # rows per input matrix
R = 2 * B                # 8192 rows of Z
D = 256                  # embedding dim
BLK = R // N_CORES       # 1024 rows per core
P = 128                  # SBUF partitions
N_CHUNKS = R // P        # 64 row-chunks
GROUPS = 8               # preamble pipeline groups
CPG = N_CHUNKS // GROUPS  # 8 chunks per group
PARTNER_OFF = B // P     # partner rows start 4096 rows (32 chunks) in
M_CHUNKS = BLK // P      # 8 stationary chunks per core
GPAIRS = 4               # column gpairs of 2048 in the main loop
NT = 512                 # matmul moving width (one PSUM bank)
S8 = 8.0                 # fp8 scale: zf8 = 8 * z / ||z||
EXP_SCALE = 5.0 / (S8 * S8)    # psum = 64*sim -> exp(psum * 5/64)
EXP_DIAG = float(np.exp(5.0))  # self-similarity term (unit rows)

F32 = mybir.dt.float32
BF16 = mybir.dt.bfloat16
FP8 = mybir.dt.float8e4
U16 = mybir.dt.uint16
AX = mybir.AxisListType
ALU = mybir.AluOpType
AF = mybir.ActivationFunctionType
PM = mybir.MatmulPerfMode

NORM_MODE = "quake"

# Schraudolph fast-exp constants for the DVE-offloaded tiles:
# i32 = psum*EXP_SCALE*2^23/ln2 + (127*2^23 - C); bitcast(i32) ~ exp(psum*EXP_SCALE)
# C calibrated for zero-mean relative error over the sim distribution.
SCH_A = EXP_SCALE * (2.0 ** 7) / float(np.log(2.0))
SCH_B = 127.0 * 2.0 ** 7 - 477742.0 / 65536.0
# (gp, m) tiles whose exp runs on DVE instead of ACT (gp>=2: DVE has
# finished the normalize pipeline by then)
DVE_TILES = {(3, 1), (3, 3), (3, 5), (3, 7)}

_prog = None


def _patch_act_tables():
    """Make natural_log_exp_and_others the only provider of Exp/Ln so the
    table-load pass emits ONE load (ids are positional; membership edits
    don't change ids)."""
    if getattr(bacc, "_act_tables_patched", False):
        return
    orig = bacc.get_activation_tables

    def patched(arch):
        t = orig(arch)
        for name, funcs in t.items():
            if name != "natural_log_exp_and_others":
                funcs.discard(AF.Exp)
                funcs.discard(AF.Ln)
        return t

    bacc.get_activation_tables = patched
    bacc._act_tables_patched = True


def _build():
    _patch_act_tables()
    nc = bacc.Bacc(
        "TRN2", target_bir_lowering=False, debug=False, num_devices=N_CORES
    )
    x = nc.dram_tensor("x", [R, D], F32, kind="ExternalInput").ap()
    out = nc.dram_tensor("out", [1, 1], F32, kind="ExternalOutput").ap()

    with tile.TileContext(nc) as tc:
        with tc.tile_pool(name="big", bufs=1) as big, \
             tc.tile_pool(name="small", bufs=1) as small, \
             tc.tile_pool(name="sq", bufs=4) as sqp, \
             tc.tile_pool(name="esc", bufs=2) as esc, \
             tc.tile_pool(name="sch", bufs=2) as sch, \
             tc.tile_pool(name="psum", bufs=2, space="PSUM") as psum:

            raw = big.tile([P, N_CHUNKS, D], F32)      # 64 KiB/part
            znat8 = big.tile([P, N_CHUNKS, D], FP8)    # 16 KiB/part
            ztp = big.tile([P, R, 2], FP8)             # 16 KiB/part
            ztm = big.tile([P, 2, BLK], FP8)           # k-major stationary

            znat16 = znat8.bitcast(U16)                # [P, N_CHUNKS, 128]
            ztp16 = ztp.bitcast(U16)                   # [P, R]

            ss = small.tile([P, N_CHUNKS], F32)        # row sums of squares
            inv8 = small.tile([P, N_CHUNKS], F32)      # 8/row-norm
            dotraw = small.tile([P, M_CHUNKS], F32)    # raw pos dot products
            posb = small.tile([P, M_CHUNKS], F32)
            acc = small.tile([P, M_CHUNKS, GPAIRS], F32)
            rows = small.tile([P, M_CHUNKS], F32)      # denominators
            lnd = small.tile([P, M_CHUNKS], F32)
            comb = small.tile([P, M_CHUNKS], F32)
            partial = small.tile([P, 1], F32)
            ones = small.tile([P, 1], F32)
            outsb = small.tile([1, 1], F32)

            nc.vector.memset(ones, 1.0 / float(R))

            if NORM_MODE == "quake":
                q_i32 = small.tile([P, N_CHUNKS], mybir.dt.int32)
                q_f32 = q_i32.bitcast(F32)
                nwt = small.tile([P, N_CHUNKS], F32)

            # ---- preamble: load, norms, inv8, normalize, pair-transpose ----
            # 5 consolidated loads on gpsimd (group order = earliest groups
            # get full HBM bandwidth; few DMA instructions keep the DMA
            # semaphore pool from serializing transposes behind loads)
            for lo, hi in ((0, 1), (1, 2), (2, 4), (4, 6), (6, 8)):
                c0 = lo * CPG
                r0 = c0 * P
                n = (hi - lo) * CPG
                nc.gpsimd.dma_start(
                    out=raw[:, c0:c0 + n, :],
                    in_=x[r0:r0 + n * P, :].rearrange("(c p) d -> p c d", p=P),
                )
            for g in range(GROUPS):
                c0 = g * CPG
                gs = slice(c0, c0 + CPG)
                r0 = c0 * P
                for ci in range(c0, c0 + CPG):
                    sqt = sqp.tile([P, D], BF16)
                    nc.vector.scalar_tensor_tensor(
                        out=sqt, in0=raw[:, ci, :], scalar=1.0,
                        in1=raw[:, ci, :],
                        op0=ALU.mult, op1=ALU.mult,
                        accum_out=ss[:, ci:ci + 1],
                    )
                if NORM_MODE == "pow":
                    # inv8 = 8 * ss^-0.5 in one DVE pass
                    nc.vector.tensor_scalar(
                        out=inv8[:, gs], in0=ss[:, gs],
                        scalar1=-0.5, scalar2=S8, op0=ALU.pow, op1=ALU.mult,
                    )
                else:
                    # quake rsqrt seed + 2 Newton iterations, all DVE
                    ss_i32 = ss.bitcast(mybir.dt.int32)
                    # seed = 0x5F3759DF - (i >> 1); bitwise and arith ops
                    # cannot mix in one tensor_scalar.
                    nc.vector.tensor_scalar(
                        out=q_i32[:, gs], in0=ss_i32[:, gs],
                        scalar1=1, scalar2=None,
                        op0=ALU.logical_shift_right,
                    )
                    nc.vector.tensor_scalar(
                        out=q_i32[:, gs], in0=q_i32[:, gs],
                        scalar1=0x5F3759DF, scalar2=-1,
                        op0=ALU.subtract, op1=ALU.mult,
                    )
                    for _ in range(1):
                        # y <- y * (1.5 - 0.5*ss*y^2)
                        nc.vector.tensor_tensor(
                            out=nwt[:, gs], in0=q_f32[:, gs], in1=q_f32[:, gs],
                            op=ALU.mult,
                        )
                        nc.vector.tensor_tensor(
                            out=nwt[:, gs], in0=nwt[:, gs], in1=ss[:, gs],
                            op=ALU.mult,
                        )
                        nc.vector.tensor_scalar(
                            out=nwt[:, gs], in0=nwt[:, gs],
                            scalar1=-0.5, scalar2=1.5, op0=ALU.mult, op1=ALU.add,
                        )
                        nc.vector.tensor_tensor(
                            out=q_f32[:, gs], in0=q_f32[:, gs], in1=nwt[:, gs],
                            op=ALU.mult,
                        )
                    nc.vector.tensor_scalar(
                        out=inv8[:, gs], in0=q_f32[:, gs],
                        scalar1=S8, scalar2=None, op0=ALU.mult,
                    )
                for ci in range(c0, c0 + CPG):
                    nc.vector.tensor_scalar_mul(
                        znat8[:, ci, :], raw[:, ci, :], inv8[:, ci:ci + 1]
                    )
                if g % 2 == 1:
                    # one transpose per gpair (groups g-1, g): exactly the
                    # column range each main-loop gpair consumes
                    c0p = (g - 1) * CPG
                    rp = c0p * P
                    nc.sync.dma_start_transpose(
                        out=ztp16[:, rp:rp + 2 * CPG * P, :].rearrange(
                            "q (a p) o -> q a (p o)", p=P
                        ),
                        in_=znat16[:, c0p:c0p + 2 * CPG, :],
                    )
                if g == 1:
                    # k-major copy of own rows for ldweights (the pair-
                    # interleaved layout violates s3_lw_dual_fp8 rules)
                    for i in range(2):
                        nc.vector.tensor_copy(
                            ztm[:, i, :], ztp[:, 0:BLK, i]
                        )

            # ---- positives: pos_r = (e_r . e_{r+B}) raw fp32 ----
            for j in range(M_CHUNKS):
                pscr = sqp.tile([P, D], BF16)
                nc.vector.scalar_tensor_tensor(
                    out=pscr, in0=raw[:, j, :], scalar=1.0,
                    in1=raw[:, j + PARTNER_OFF, :],
                    op0=ALU.mult, op1=ALU.mult,
                    accum_out=dotraw[:, j:j + 1],
                )
            nc.vector.tensor_mul(posb, dotraw, inv8[:, 0:M_CHUNKS])
            nc.vector.tensor_mul(
                posb, posb, inv8[:, PARTNER_OFF:PARTNER_OFF + M_CHUNKS]
            )

            # ---- main loop: DoubleRow matmuls + exp row-sums ----
            for gp in range(GPAIRS):
                for m in range(M_CHUNKS):
                    pt = psum.tile([P, 4 * NT], F32)
                    lhsT = ztm[:, :, m * P:(m + 1) * P]
                    for b in range(4):
                        col = gp * (4 * NT) + b * NT
                        nc.tensor.matmul(
                            pt[:, b * NT:(b + 1) * NT],
                            lhsT,
                            ztp[:, col:col + NT, :].rearrange("p c i -> p i c"),
                            start=True,
                            stop=True,
                            perf_mode=PM.DoubleRow,
                        )
                    if (gp, m) in DVE_TILES:
                        # Schraudolph fast exp on DVE: affine into int32,
                        # bitcast back as fp32 ~ exp, then sum.
                        q16 = sch.tile([P, 4 * NT], mybir.dt.int16)
                        nc.vector.tensor_scalar(
                            out=q16, in0=pt,
                            scalar1=SCH_A, scalar2=SCH_B,
                            op0=ALU.mult, op1=ALU.add,
                        )
                        exf = esc.tile([P, 4 * NT], BF16)
                        nc.vector.tensor_scalar(
                            out=exf, in0=q16.bitcast(BF16),
                            scalar1=1.0, scalar2=None,
                            op0=ALU.mult, op1=ALU.add,
                            accum_out=acc[:, m, gp:gp + 1],
                        )
                    else:
                        ex = esc.tile([P, 4 * NT], BF16)
                        nc.scalar.activation(
                            ex,
                            pt,
                            AF.Exp,
                            scale=EXP_SCALE,
                            accum_out=acc[:, m, gp:gp + 1],
                        )

            # ---- finalize ----
            for m in range(M_CHUNKS):
                nc.vector.tensor_reduce(
                    rows[:, m:m + 1], acc[:, m, :], axis=AX.X, op=ALU.add
                )
            nc.vector.tensor_scalar_add(rows, rows, -EXP_DIAG)
            nc.scalar.activation(lnd, rows, AF.Ln)
            # comb = ln(denom) - 5*pos ; pos = dotraw*inv8_r*inv8_p/64
            nc.vector.scalar_tensor_tensor(
                out=comb,
                in0=posb,
                scalar=-5.0 / (S8 * S8),
                in1=lnd,
                op0=ALU.mult,
                op1=ALU.add,
                accum_out=partial,
            )
            fin = psum.tile([P, 4 * NT], F32, tag="pt")
            nc.tensor.matmul(
                fin[0:1, 0:1], partial, ones, start=True, stop=True
            )
            nc.vector.tensor_copy(outsb, fin[0:1, 0:1])
            nc.sync.dma_start(out=out, in_=outsb)

    nc.compile()
    return nc


def _get_prog():
    global _prog
    if _prog is None:
        _prog = _build()
    return _prog


def kernel(emb_i: np.ndarray, emb_j: np.ndarray) -> np.ndarray:
    nc = _get_prog()
    z = np.concatenate(
        [np.asarray(emb_i, np.float32), np.asarray(emb_j, np.float32)], axis=0
    )
    in_maps = [
        {"x": np.ascontiguousarray(np.roll(z, -c * BLK, axis=0))}
        for c in range(N_CORES)
    ]
    res = bass_utils.run_bass_kernel_spmd(
        nc, in_maps, core_ids=list(range(N_CORES))
    )
    total = sum(float(res.results[c]["out"][0, 0]) for c in range(N_CORES))
    return np.asarray(total, dtype=np.float32)
